# revision 1
# baseline (speedup 1.0000x reference)
"""Trainium2 Bass kernel: embedding gather + 2-layer MLP (relu), data-parallel on 8 cores.

Reference computation:
    x   = entity_embedding[idx0, idx1, :]        # [B, 128]  gather
    h   = relu(x @ w1.T + b1)                    # [B, 256]
    out = relu(h @ w2.T + b2)                    # [B, 86]

Shapes (hardcoded): entity_embedding [500000, 4, 128] f32, B = 131072.

Strategy:
  - Flatten the table to [2e6, 128]; flat row index = idx0*4 + idx1 (fits int32).
  - Shard the batch 8 ways (16384 rows/core); replicate table + weights.
  - Per core: gather rows via gpsimd indirect DMA into [128, j, 128] SBUF tiles
    (batch on partitions), transpose 128x128 sub-tiles on TensorE so features
    land on partitions, then run the MLP with batch on the free dim:
        hT[256h, b]  = w1 @ xT  (2 matmuls, N=512 free)
        outT[86, b]  = w2 @ hT  (2 accumulating matmuls)
    Biases are per-partition vectors in this orientation; relu+bias fuse into
    one ACT/DVE op per tile. Output is written transposed [86, 16384] per core
    and un-transposed on the host during unshard.
"""

import numpy as np
from contextlib import ExitStack

import concourse.bass as bass
import concourse.bacc as bacc
import concourse.tile as tile
from concourse import mybir
from concourse.bass_utils import run_bass_kernel_spmd
from concourse.masks import make_identity

F32 = mybir.dt.float32
I32 = mybir.dt.int32

N_CORES = 8
B = 131072
BC = B // N_CORES          # 16384 batch rows per core
FEAT = 128
NHID = 256
NOUT = 86
NROWS = 500000 * 4         # flattened table rows
P = 128
NJ = BC // P               # 128 j-columns of gathered rows per core
NJG = 16                   # j-columns per indirect-DMA gather call (2048 rows)
CHUNK_J = 4                # j-columns per MLP chunk (512 batch)
NCHUNK = NJ // CHUNK_J     # 32 chunks per core


def _build_program():
    nc = bacc.Bacc("TRN2", num_devices=N_CORES, num_swdge_queues=4)

    table = nc.dram_tensor("table", [NROWS, FEAT], F32, kind="ExternalInput").ap()
    idxs = nc.dram_tensor("idxs", [P, NJ], I32, kind="ExternalInput").ap()
    w1t = nc.dram_tensor("w1t", [FEAT, NHID], F32, kind="ExternalInput").ap()
    w2t = nc.dram_tensor("w2t", [NHID, NOUT], F32, kind="ExternalInput").ap()
    b1v = nc.dram_tensor("b1v", [NHID], F32, kind="ExternalInput").ap()
    b2v = nc.dram_tensor("b2v", [NOUT], F32, kind="ExternalInput").ap()
    outT = nc.dram_tensor("outT", [NOUT, BC], F32, kind="ExternalOutput").ap()

    with tile.TileContext(nc) as tc, ExitStack() as ctx:
        const = ctx.enter_context(tc.tile_pool(name="const", bufs=1))
        gpool = ctx.enter_context(tc.tile_pool(name="gather", bufs=4))
        xpool = ctx.enter_context(tc.tile_pool(name="xt", bufs=3))
        hpool = ctx.enter_context(tc.tile_pool(name="ht", bufs=3))
        opool = ctx.enter_context(tc.tile_pool(name="ot", bufs=3))
        psum = ctx.enter_context(tc.tile_pool(name="psum", bufs=2, space="PSUM"))

        idt = const.tile([P, P], F32)
        make_identity(nc, idt[:])

        w1t_t = const.tile([FEAT, NHID], F32)
        nc.sync.dma_start(w1t_t[:], w1t[:])
        w2t_t = const.tile([P, NHID // P, NOUT], F32)
        nc.sync.dma_start(w2t_t[:], w2t.rearrange("(k p) n -> p k n", p=P))
        b1_t = const.tile([P, NHID // P], F32)
        nc.sync.dma_start(b1_t[:], b1v.rearrange("(k p) -> p k", p=P))
        b2_t = const.tile([NOUT, 1], F32)
        nc.sync.dma_start(b2_t[:], b2v.rearrange("(n one) -> n one", one=1))
        idx_t = const.tile([P, NJ], I32)
        nc.sync.dma_start(idx_t[:], idxs[:])

        for c in range(NCHUNK):
            bcol = c * CHUNK_J * P  # column base in outT for this chunk
            # HW indirect DMA consumes ONE index per partition; gather the
            # chunk's 512 rows as CHUNK_J calls of 128 rows each.
            gt = gpool.tile([P, CHUNK_J, FEAT], F32)
            for i in range(CHUNK_J):
                j = c * CHUNK_J + i
                ginst = nc.gpsimd.indirect_dma_start(
                    out=gt[:, i, :],
                    out_offset=None,
                    in_=table[:],
                    in_offset=bass.IndirectOffsetOnAxis(
                        ap=idx_t[:, j:j + 1], axis=0
                    ),
                )
                # Spread descriptor generation over the 4 SWDGE queues
                # (parallel Q7 pairs + separate DMA rings).
                ginst.ins.queue = f"qPoolDynamic{i or ''}"
            if True:
                # Transpose 4x [128b, 128f] -> [128f, 128b] into one PSUM bank
                xtr = psum.tile([P, CHUNK_J * P], F32, tag="xtr")
                for i in range(CHUNK_J):
                    nc.tensor.transpose(
                        out=xtr[:, i * P:(i + 1) * P],
                        in_=gt[:, i, :],
                        identity=idt[:],
                    )
                xt = xpool.tile([P, CHUNK_J * P], F32)
                nc.vector.tensor_copy(out=xt[:], in_=xtr[:])

                # lin1: hT[k] = relu(w1[k] @ xT + b1[k]), k over 2 hid halves
                ht = hpool.tile([P, NHID // P, CHUNK_J * P], F32)
                for k in range(NHID // P):
                    hp = psum.tile([P, CHUNK_J * P], F32, tag=f"h{k}")
                    nc.tensor.matmul(
                        out=hp[:],
                        lhsT=w1t_t[:, k * P:(k + 1) * P],
                        rhs=xt[:],
                        start=True,
                        stop=True,
                    )
                    if k == 0:
                        nc.scalar.activation(
                            out=ht[:, k, :], in_=hp[:],
                            func=mybir.ActivationFunctionType.Relu,
                            bias=b1_t[:, k:k + 1],
                        )
                    else:
                        nc.vector.tensor_scalar(
                            out=ht[:, k, :], in0=hp[:],
                            scalar1=b1_t[:, k:k + 1], scalar2=0.0,
                            op0=mybir.AluOpType.add, op1=mybir.AluOpType.max,
                        )

                # lin2: outT = relu(w2 @ hT + b2), accumulate over 2 k-tiles
                op_ = psum.tile([NOUT, CHUNK_J * P], F32, tag="ot")
                for k in range(NHID // P):
                    nc.tensor.matmul(
                        out=op_[:],
                        lhsT=w2t_t[:, k, :],
                        rhs=ht[:, k, :],
                        start=(k == 0),
                        stop=(k == NHID // P - 1),
                    )
                ot = opool.tile([NOUT, CHUNK_J * P], F32)
                nc.scalar.activation(
                    out=ot[:], in_=op_[:],
                    func=mybir.ActivationFunctionType.Relu,
                    bias=b2_t[:],
                )
                nc.sync.dma_start(outT[:, bcol:bcol + CHUNK_J * P], ot[:])

    nc.compile()
    return nc


TRACE = False          # set by test harness to capture an NTFF profile
RUN_KWARGS = None      # extra kwargs for run_bass_kernel_spmd (test harness)
LAST = None            # last BassKernelResults (test harness reads exec_time_ns)

_SLOT_TO_BATCH = None


def _slot_map():
    """local batch index for gather slot (p, j): out column c*512 + i*128 + p
    where j = c*CHUNK_J + i must equal the local batch index."""
    global _SLOT_TO_BATCH
    if _SLOT_TO_BATCH is None:
        pp, jj = np.meshgrid(np.arange(P), np.arange(NJ), indexing="ij")
        cc = jj // CHUNK_J
        ii = jj % CHUNK_J
        _SLOT_TO_BATCH = cc * (CHUNK_J * P) + ii * P + pp  # [P, NJ]
    return _SLOT_TO_BATCH


def kernel(entity_embedding, w1, b1, w2, b2, idx0, idx1):
    table = np.ascontiguousarray(
        np.asarray(entity_embedding, dtype=np.float32).reshape(NROWS, FEAT)
    )
    flat_idx = (np.asarray(idx0, dtype=np.int64) * 4
                + np.asarray(idx1, dtype=np.int64)).astype(np.int32)
    w1t = np.ascontiguousarray(np.asarray(w1, dtype=np.float32).T)
    w2t = np.ascontiguousarray(np.asarray(w2, dtype=np.float32).T)
    b1v = np.ascontiguousarray(np.asarray(b1, dtype=np.float32))
    b2v = np.ascontiguousarray(np.asarray(b2, dtype=np.float32))

    slot = _slot_map()
    in_maps = []
    for core in range(N_CORES):
        local = flat_idx[core * BC:(core + 1) * BC]
        idxs = np.ascontiguousarray(local[slot])  # [P, NJ] int32
        in_maps.append({
            "table": table,
            "idxs": idxs,
            "w1t": w1t,
            "w2t": w2t,
            "b1v": b1v,
            "b2v": b2v,
        })

    nc = _build_program()
    global LAST
    res = run_bass_kernel_spmd(
        nc, in_maps, core_ids=list(range(N_CORES)), trace=TRACE,
        **(RUN_KWARGS or {}),
    )
    LAST = res
    out = np.empty((B, NOUT), dtype=np.float32)
    for core in range(N_CORES):
        out[core * BC:(core + 1) * BC] = res.results[core]["outT"].T
    return out


if __name__ == "__main__":
    rng = np.random.default_rng(0)
    ins = {
        "entity_embedding": rng.standard_normal((500000, 4, FEAT), dtype=np.float32),
        "w1": rng.standard_normal((NHID, FEAT), dtype=np.float32) / np.sqrt(FEAT),
        "b1": rng.standard_normal((NHID,), dtype=np.float32) / np.sqrt(FEAT),
        "w2": rng.standard_normal((NOUT, NHID), dtype=np.float32) / np.sqrt(NHID),
        "b2": rng.standard_normal((NOUT,), dtype=np.float32) / np.sqrt(NHID),
        "idx0": rng.integers(0, 500000, B).astype(np.int32),
        "idx1": rng.integers(0, 4, B).astype(np.int32),
    }
    out = kernel(**ins)
    x = ins["entity_embedding"].reshape(NROWS, FEAT)[
        ins["idx0"].astype(np.int64) * 4 + ins["idx1"]]
    h = np.maximum(x @ ins["w1"].T + ins["b1"], 0.0)
    ref = np.maximum(h @ ins["w2"].T + ins["b2"], 0.0)
    err = np.abs(out - ref).max() / max(np.abs(ref).max(), 1e-9)
    print("rel err:", err)



# revision 13
# speedup vs baseline: 1.0258x; 1.0258x over previous
"""Trainium2 Bass kernel: embedding gather + 2-layer MLP (relu), data-parallel on 8 cores.

Reference computation:
    x   = entity_embedding[idx0, idx1, :]        # [B, 128]  gather
    h   = relu(x @ w1.T + b1)                    # [B, 256]
    out = relu(h @ w2.T + b2)                    # [B, 86]

Shapes (hardcoded): entity_embedding [500000, 4, 128] f32, B = 131072.

Strategy:
  - Flatten the table to [2e6, 128]; flat row index = idx0*4 + idx1 (fits int32).
  - Shard the batch 8 ways (16384 rows/core); replicate table + weights.
  - Per core: gather rows via gpsimd indirect DMA into [128, j, 128] SBUF tiles
    (batch on partitions), transpose 128x128 sub-tiles on TensorE so features
    land on partitions, then run the MLP with batch on the free dim:
        hT[256h, b]  = w1 @ xT  (2 matmuls, N=512 free)
        outT[86, b]  = w2 @ hT  (2 accumulating matmuls)
    The MLP matmuls run in bf16 (1 cycle/row on the PE instead of 4 for fp32);
    weights are converted host-side, x and h are rounded to bf16 for free
    inside the PSUM->SBUF copy / activation ops that must run anyway.
    Biases are per-partition vectors in this orientation; relu+bias fuse into
    one ACT/DVE op per tile. Output is written transposed [86, 16384] per core
    and un-transposed on the host during unshard.
"""

import numpy as np
from contextlib import ExitStack

import concourse.bass as bass
import concourse.bacc as bacc
import concourse.tile as tile
from concourse import mybir
from concourse.bass_utils import run_bass_kernel_spmd
from concourse.masks import make_identity

F32 = mybir.dt.float32
BF16 = mybir.dt.bfloat16
I32 = mybir.dt.int32

N_CORES = 8
B = 131072
BC = B // N_CORES          # 16384 batch rows per core
FEAT = 128
NHID = 256
NOUT = 86
NROWS = 500000 * 4         # flattened table rows
P = 128
NJ = BC // P               # 128 j-columns of gathered rows per core
CHUNK_J = 4                # j-columns per MLP chunk (512 batch)
NCHUNK = NJ // CHUNK_J     # 32 chunks per core


def _build_program():
    nc = bacc.Bacc("TRN2", num_devices=N_CORES, num_swdge_queues=4)

    table = nc.dram_tensor("table", [NROWS, FEAT], F32, kind="ExternalInput").ap()
    idxs = nc.dram_tensor("idxs", [P, NJ], I32, kind="ExternalInput").ap()
    w1t = nc.dram_tensor("w1t", [FEAT, NHID], BF16, kind="ExternalInput").ap()
    w2t = nc.dram_tensor("w2t", [NHID, NOUT], BF16, kind="ExternalInput").ap()
    b1v = nc.dram_tensor("b1v", [NHID], F32, kind="ExternalInput").ap()
    b2v = nc.dram_tensor("b2v", [NOUT], F32, kind="ExternalInput").ap()
    outT = nc.dram_tensor("outT", [NOUT, BC], F32, kind="ExternalOutput").ap()

    with tile.TileContext(nc) as tc, ExitStack() as ctx:
        const = ctx.enter_context(tc.tile_pool(name="const", bufs=1))
        gpool = ctx.enter_context(tc.tile_pool(name="gather", bufs=4))
        xpool = ctx.enter_context(tc.tile_pool(name="xt", bufs=3))
        hpool = ctx.enter_context(tc.tile_pool(name="ht", bufs=3))
        opool = ctx.enter_context(tc.tile_pool(name="ot", bufs=3))
        psum = ctx.enter_context(tc.tile_pool(name="psum", bufs=2, space="PSUM"))

        idt = const.tile([P, P], F32)
        make_identity(nc, idt[:])

        w1t_t = const.tile([FEAT, NHID], BF16)
        nc.sync.dma_start(w1t_t[:], w1t[:])
        w2t_t = const.tile([P, NHID // P, NOUT], BF16)
        nc.sync.dma_start(w2t_t[:], w2t.rearrange("(k p) n -> p k n", p=P))
        b1_t = const.tile([P, NHID // P], F32)
        nc.sync.dma_start(b1_t[:], b1v.rearrange("(k p) -> p k", p=P))
        b2_t = const.tile([NOUT, 1], F32)
        nc.sync.dma_start(b2_t[:], b2v.rearrange("(n one) -> n one", one=1))
        idx_t = const.tile([P, NJ], I32)
        nc.sync.dma_start(idx_t[:], idxs[:])

        for c in range(NCHUNK):
            bcol = c * CHUNK_J * P  # column base in outT for this chunk
            # HW indirect DMA consumes ONE index per partition; gather the
            # chunk's 512 rows as CHUNK_J calls of 128 rows each.
            gt = gpool.tile([P, CHUNK_J, FEAT], F32)
            for i in range(CHUNK_J):
                j = c * CHUNK_J + i
                ginst = nc.gpsimd.indirect_dma_start(
                    out=gt[:, i, :],
                    out_offset=None,
                    in_=table[:],
                    in_offset=bass.IndirectOffsetOnAxis(
                        ap=idx_t[:, j:j + 1], axis=0
                    ),
                )
                # Spread descriptor generation over the 4 SWDGE queues
                # (parallel Q7 pairs + separate DMA rings).
                ginst.ins.queue = f"qPoolDynamic{i or ''}"
            if True:
                # Transpose 4x [128b, 128f] -> [128f, 128b] into one PSUM bank
                xtr = psum.tile([P, CHUNK_J * P], F32, tag="xtr")
                for i in range(CHUNK_J):
                    nc.tensor.transpose(
                        out=xtr[:, i * P:(i + 1) * P],
                        in_=gt[:, i, :],
                        identity=idt[:],
                    )
                xt = xpool.tile([P, CHUNK_J * P], BF16)
                nc.vector.tensor_copy(out=xt[:], in_=xtr[:])

                # lin1: hT[k] = relu(w1[k] @ xT + b1[k]), k over 2 hid halves
                ht = hpool.tile([P, NHID // P, CHUNK_J * P], BF16)
                for k in range(NHID // P):
                    hp = psum.tile([P, CHUNK_J * P], F32, tag=f"h{k}")
                    nc.tensor.matmul(
                        out=hp[:],
                        lhsT=w1t_t[:, k * P:(k + 1) * P],
                        rhs=xt[:],
                        start=True,
                        stop=True,
                    )
                    if k == 0:
                        nc.scalar.activation(
                            out=ht[:, k, :], in_=hp[:],
                            func=mybir.ActivationFunctionType.Relu,
                            bias=b1_t[:, k:k + 1],
                        )
                    else:
                        nc.vector.tensor_scalar(
                            out=ht[:, k, :], in0=hp[:],
                            scalar1=b1_t[:, k:k + 1], scalar2=0.0,
                            op0=mybir.AluOpType.add, op1=mybir.AluOpType.max,
                        )

                # lin2: outT = relu(w2 @ hT + b2), accumulate over 2 k-tiles
                op_ = psum.tile([NOUT, CHUNK_J * P], F32, tag="ot")
                for k in range(NHID // P):
                    nc.tensor.matmul(
                        out=op_[:],
                        lhsT=w2t_t[:, k, :],
                        rhs=ht[:, k, :],
                        start=(k == 0),
                        stop=(k == NHID // P - 1),
                    )
                ot = opool.tile([NOUT, CHUNK_J * P], F32)
                nc.scalar.activation(
                    out=ot[:], in_=op_[:],
                    func=mybir.ActivationFunctionType.Relu,
                    bias=b2_t[:],
                )
                nc.sync.dma_start(outT[:, bcol:bcol + CHUNK_J * P], ot[:])

    nc.compile()
    return nc


TRACE = False          # set by test harness to capture an NTFF profile
RUN_KWARGS = None      # extra kwargs for run_bass_kernel_spmd (test harness)
LAST = None            # last BassKernelResults (test harness reads exec_time_ns)

_SLOT_TO_BATCH = None


def _slot_map():
    """local batch index for gather slot (p, j): out column c*512 + i*128 + p
    where j = c*CHUNK_J + i must equal the local batch index."""
    global _SLOT_TO_BATCH
    if _SLOT_TO_BATCH is None:
        pp, jj = np.meshgrid(np.arange(P), np.arange(NJ), indexing="ij")
        cc = jj // CHUNK_J
        ii = jj % CHUNK_J
        _SLOT_TO_BATCH = cc * (CHUNK_J * P) + ii * P + pp  # [P, NJ]
    return _SLOT_TO_BATCH


def _to_bf16_bits(a):
    """Round-to-nearest-even f32 -> bf16, returned as uint16 view for upload."""
    import ml_dtypes
    return a.astype(ml_dtypes.bfloat16)


def kernel(entity_embedding, w1, b1, w2, b2, idx0, idx1):
    table = np.ascontiguousarray(
        np.asarray(entity_embedding, dtype=np.float32).reshape(NROWS, FEAT)
    )
    flat_idx = (np.asarray(idx0, dtype=np.int64) * 4
                + np.asarray(idx1, dtype=np.int64)).astype(np.int32)
    w1t = _to_bf16_bits(np.ascontiguousarray(np.asarray(w1, dtype=np.float32).T))
    w2t = _to_bf16_bits(np.ascontiguousarray(np.asarray(w2, dtype=np.float32).T))
    b1v = np.ascontiguousarray(np.asarray(b1, dtype=np.float32))
    b2v = np.ascontiguousarray(np.asarray(b2, dtype=np.float32))

    slot = _slot_map()
    in_maps = []
    for core in range(N_CORES):
        local = flat_idx[core * BC:(core + 1) * BC]
        idxs = np.ascontiguousarray(local[slot])  # [P, NJ] int32
        in_maps.append({
            "table": table,
            "idxs": idxs,
            "w1t": w1t,
            "w2t": w2t,
            "b1v": b1v,
            "b2v": b2v,
        })

    nc = _build_program()
    global LAST
    res = run_bass_kernel_spmd(
        nc, in_maps, core_ids=list(range(N_CORES)), trace=TRACE,
        **(RUN_KWARGS or {}),
    )
    LAST = res
    out = np.empty((B, NOUT), dtype=np.float32)
    for core in range(N_CORES):
        out[core * BC:(core + 1) * BC] = res.results[core]["outT"].T
    return out


if __name__ == "__main__":
    rng = np.random.default_rng(0)
    ins = {
        "entity_embedding": rng.standard_normal((500000, 4, FEAT), dtype=np.float32),
        "w1": rng.standard_normal((NHID, FEAT), dtype=np.float32) / np.sqrt(FEAT),
        "b1": rng.standard_normal((NHID,), dtype=np.float32) / np.sqrt(FEAT),
        "w2": rng.standard_normal((NOUT, NHID), dtype=np.float32) / np.sqrt(NHID),
        "b2": rng.standard_normal((NOUT,), dtype=np.float32) / np.sqrt(NHID),
        "idx0": rng.integers(0, 500000, B).astype(np.int32),
        "idx1": rng.integers(0, 4, B).astype(np.int32),
    }
    out = kernel(**ins)
    x = ins["entity_embedding"].reshape(NROWS, FEAT)[
        ins["idx0"].astype(np.int64) * 4 + ins["idx1"]]
    h = np.maximum(x @ ins["w1"].T + ins["b1"], 0.0)
    ref = np.maximum(h @ ins["w2"].T + ins["b2"], 0.0)
    err = np.abs(out - ref).max() / max(np.abs(ref).max(), 1e-9)
    print("rel err:", err)
